# revision 12
# baseline (speedup 1.0000x reference)
"""MoE router kernel for Trainium2 (8 NeuronCores, token-parallel).

Math: logits = hidden @ W.T ([16384, 64]); top-2 selection; combine
weights = renormalized top-2 softmax probs (softmax denominator cancels:
w2 = sigmoid(l2 - l1), w1 = 1 - w2); dispatch_mask = one-hot of the
selected experts; aux_loss = AUX_W/n * sum_e S_e^2 with
S_e = sum_tokens(router_prob_per_expert).

Precision: hidden/W are split on host into fp16 hi + fp16 lo
(x = hi + lo exact to ~2^-24), and the device computes
  logits = hi_h @ (W_hi + W_lo) + lo_h @ W_hi    (fp32 PSUM accumulation)
which gives |err| < ~6e-6, far below the minimum top-2/top-3 logit
margin of this problem size (~2.2e-5), so selections match a full-fp32
reference exactly. Two PE streams instead of three: the stationary
operand for stream 1 is the 128-wide concat [W_hi | W_lo].

Sharding: tokens (batch*seq = 16384) split 8 ways; W replicated.
Aux-loss partial sums ([128, 64] per core) are reduced on host.
"""

import sys

sys.path.insert(0, "/opt/trn_rl_repo")

import numpy as np

B, S, H, E, K = 4, 4096, 4096, 64, 2
AUX_W = 0.01
NCORES = 8
N = B * S            # 16384 tokens
T = N // NCORES      # 2048 tokens per core
NTB = 4              # t-blocks per core
TB = T // NTB        # 512 tokens per t-block
NH = H // 128        # 32 h-tiles
NSLICE = T // 128    # 16 token-slices per core

_CACHE = {}


def _build():
    import concourse.bacc as bacc
    import concourse.tile as tile
    from concourse import mybir
    from concourse.masks import make_identity

    nc = bacc.Bacc("TRN2", target_bir_lowering=False, debug=False)
    f16, f32, u32 = mybir.dt.float16, mybir.dt.float32, mybir.dt.uint32

    hh = nc.dram_tensor("hh", [H, T], f16, kind="ExternalInput")
    hl = nc.dram_tensor("hl", [H, T], f16, kind="ExternalInput")
    wcat = nc.dram_tensor("wcat", [H, 128], f16, kind="ExternalInput")
    iota = nc.dram_tensor("iota", [128, E], f32, kind="ExternalInput")
    disp = nc.dram_tensor("disp", [T, 128], f32, kind="ExternalOutput")
    comb = nc.dram_tensor("comb", [2 * NSLICE, 128], f32, kind="ExternalOutput")
    accd = nc.dram_tensor("accd", [128, E], f32, kind="ExternalOutput")

    hh_r = hh.ap().rearrange("(i p) t -> p i t", p=128)
    hl_r = hl.ap().rearrange("(i p) t -> p i t", p=128)

    with tile.TileContext(nc) as tc:
        with (
            tc.tile_pool(name="setup", bufs=1) as setup,
            tc.tile_pool(name="hbuf", bufs=2) as hbuf,
            tc.tile_pool(name="ps13", bufs=2, space="PSUM") as ps13,
            tc.tile_pool(name="ps2", bufs=2, space="PSUM") as ps2,
            tc.tile_pool(name="psT", bufs=2, space="PSUM") as psT,
            tc.tile_pool(name="lgp", bufs=2) as lgp,
            tc.tile_pool(name="slc", bufs=3) as slc,
            tc.tile_pool(name="dout", bufs=3) as dout,
        ):
            wcat_sb = setup.tile([128, NH, 128], f16)
            nc.sync.dma_start(wcat_sb[:], wcat.ap().rearrange("(i p) e -> p i e", p=128))
            iota_sb = setup.tile([128, E], f32)
            nc.sync.dma_start(iota_sb[:], iota.ap())
            ident = setup.tile([128, 128], f32)
            make_identity(nc, ident[:])
            # fold matrix J[r, e] = 1 iff r == e (mod 64): lgw_slice.T @ J
            # transposes the logit block AND sums the Whi/Wlo halves.
            fold = setup.tile([128, E], f32)
            nc.gpsimd.memset(fold[:], 0.0)
            nc.gpsimd.affine_select(
                out=fold[:], in_=fold[:], pattern=[[-1, E]],
                compare_op=mybir.AluOpType.not_equal, fill=1.0,
                base=0, channel_multiplier=1)
            nc.gpsimd.affine_select(
                out=fold[:], in_=fold[:], pattern=[[-1, E]],
                compare_op=mybir.AluOpType.not_equal, fill=1.0,
                base=-E, channel_multiplier=1)
            acc_sb = setup.tile([128, E], f32)
            nc.vector.memset(acc_sb[:], 0.0)
            comb_sb = setup.tile([128, NSLICE, 2], f32)

            for tb in range(NTB):
                t0 = tb * TB
                hh_sb = hbuf.tile([128, NH, TB], f16, tag="hh")
                hl_sb = hbuf.tile([128, NH, TB], f16, tag="hl")
                # split each block DMA across all 16 DMA engines; all hh
                # before hl so pass-1 matmuls start as soon as possible and
                # pass-2's data lands while pass 1 runs
                for q in range(8):
                    i0, i1 = q * 4, q * 4 + 4
                    nc.sync.dma_start(hh_sb[:, i0:i1, :], hh_r[:, i0:i1, t0:t0 + TB])
                for q in range(8):
                    i0, i1 = q * 4, q * 4 + 4
                    nc.sync.dma_start(hl_sb[:, i0:i1, :], hl_r[:, i0:i1, t0:t0 + TB])

                # rows 0:64 accumulate Whi.hh; rows 64:128 accumulate
                # Wlo.hh then (pass 2) += Whi.hl
                p13 = ps13.tile([128, TB], f32)
                for i in range(NH):
                    nc.tensor.matmul(p13[:], wcat_sb[:, i, :], hh_sb[:, i, :],
                                     start=(i == 0), stop=False)
                for i in range(NH):
                    nc.tensor.matmul(p13[64:128, :], wcat_sb[:, i, 0:E],
                                     hl_sb[:, i, :],
                                     start=False, stop=(i == NH - 1),
                                     skip_group_check=True)

                lgw = lgp.tile([128, TB], f32)
                nc.vector.tensor_copy(lgw[:], p13[:])

                for j in range(TB // 128):
                    s = tb * (TB // 128) + j
                    pT = psT.tile([128, E], f32)
                    nc.tensor.matmul(pT[:], lgw[:, j * 128:(j + 1) * 128],
                                     fold[:], start=True, stop=True)
                    lg_s = slc.tile([128, E], f32, tag="lg")
                    nc.scalar.copy(lg_s[:], pT[:])

                    mx8 = slc.tile([128, 8], f32, tag="mx")
                    idx8 = slc.tile([128, 8], u32, tag="ix")
                    idxf = slc.tile([128, 2], f32, tag="if")
                    nc.vector.max(out=mx8[:], in_=lg_s[:])
                    nc.vector.max_index(out=idx8[:], in_max=mx8[:], in_values=lg_s[:])
                    nc.vector.tensor_copy(idxf[:], idx8[:, 0:2])

                    disp_t = dout.tile([128, 128], f32)
                    nc.vector.tensor_single_scalar(
                        disp_t[:, 0:E], iota_sb[:], idxf[:, 0:1],
                        mybir.AluOpType.is_equal)
                    nc.vector.tensor_single_scalar(
                        disp_t[:, E:128], iota_sb[:], idxf[:, 1:2],
                        mybir.AluOpType.is_equal)

                    d = slc.tile([128, 1], f32, tag="d")
                    nc.vector.tensor_sub(d[:], mx8[:, 1:2], mx8[:, 0:1])
                    w2 = comb_sb[:, s, 1:2]
                    w1 = comb_sb[:, s, 0:1]
                    nc.scalar.activation(w2, d[:],
                                         mybir.ActivationFunctionType.Sigmoid)
                    nc.scalar.activation(w1, w2,
                                         mybir.ActivationFunctionType.Copy,
                                         bias=1.0, scale=-1.0)

                    # acc += dm1*w1 + dm2*w2
                    nc.vector.scalar_tensor_tensor(
                        acc_sb[:], disp_t[:, 0:E], w1, acc_sb[:],
                        op0=mybir.AluOpType.mult, op1=mybir.AluOpType.add)
                    nc.vector.scalar_tensor_tensor(
                        acc_sb[:], disp_t[:, E:128], w2, acc_sb[:],
                        op0=mybir.AluOpType.mult, op1=mybir.AluOpType.add)

                    nc.sync.dma_start(disp.ap()[s * 128:(s + 1) * 128, :], disp_t[:])

            combT = psT.tile([2 * NSLICE, 128], f32)
            nc.tensor.transpose(combT[:], comb_sb[:].rearrange("p s k -> p (s k)"),
                                ident[:])
            combT_sb = setup.tile([2 * NSLICE, 128], f32)
            nc.vector.tensor_copy(combT_sb[:], combT[:])
            nc.sync.dma_start(comb.ap(), combT_sb[:])
            nc.sync.dma_start(accd.ap(), acc_sb[:])

    nc.compile()
    return nc


def _get_nc():
    if "nc" not in _CACHE:
        _CACHE["nc"] = _build()
    return _CACHE["nc"]


def _prep_inputs(hidden_states, W):
    h = np.asarray(hidden_states, dtype=np.float32).reshape(N, H)
    Wf = np.asarray(W, dtype=np.float32)
    WT = np.ascontiguousarray(Wf.T)                       # [H, E]
    Whi = WT.astype(np.float16)
    Wlo = (WT - Whi.astype(np.float32)).astype(np.float16)
    wcat = np.ascontiguousarray(np.concatenate([Whi, Wlo], axis=1))  # [H, 128]
    iota = np.ascontiguousarray(
        np.broadcast_to(np.arange(E, dtype=np.float32), (128, E)))
    in_maps = []
    for c in range(NCORES):
        chT = np.ascontiguousarray(h[c * T:(c + 1) * T].T)  # [H, T]
        hh = chT.astype(np.float16)
        hl = (chT - hh.astype(np.float32)).astype(np.float16)
        in_maps.append({"hh": hh, "hl": hl, "wcat": wcat, "iota": iota})
    return in_maps


def _gather(results):
    disp = np.concatenate([r["disp"] for r in results], axis=0)
    dispatch_mask = disp.reshape(N, K, E)
    combs = []
    for r in results:
        ct = r["comb"].reshape(NSLICE, 2, 128)         # [s, k, p]
        combs.append(np.ascontiguousarray(ct.transpose(0, 2, 1)).reshape(T, 2))
    combine = np.concatenate(combs, axis=0).reshape(B, S, K, 1)
    Se = np.zeros(E, dtype=np.float64)
    for r in results:
        Se += r["accd"].astype(np.float64).sum(axis=0)
    aux = np.float32(AUX_W / N * float((Se ** 2).sum()))
    return dispatch_mask, combine, aux


def run(hidden_states, W, trace=False, tmpdir=None):
    from concourse import bass_utils
    nc = _get_nc()
    in_maps = _prep_inputs(hidden_states, W)
    res = bass_utils.run_bass_kernel_spmd(
        nc, in_maps, core_ids=list(range(NCORES)), trace=trace, tmpdir=tmpdir)
    return _gather(res.results), res


def kernel(hidden_states, W):
    (dispatch_mask, combine, aux), _ = run(hidden_states, W)
    return dispatch_mask, combine, aux
